# revision 18
# baseline (speedup 1.0000x reference)
"""GCN layer (GCNConv + PReLU) on 8 Trainium2 NeuronCores.

Math (equivalent to the PyG-style reference):
    h   = x @ W.T                       # [N, H] dense transform
    deg = bincount(col) + 1             # self-loops included
    dinv = 1/sqrt(deg)
    g   = h * dinv[:, None]             # fold source-side norm into nodes
    S_d = sum_{e: col_e = d} g[row_e]   # includes self edge (d, d)
    out = prelu(dinv_d * S_d + b)       # per-channel slope a

Distribution: destination nodes are grouped into 784 degree-balanced blocks
of 128 slots; each core owns 98 blocks.  Phase 1 computes g for a contiguous
node shard per core (x.T shard @ W.T on the PE, fp32), then an AllGather
replicates the full g table [100352, 128] (bf16) to every core.  Phase 2
streams each block's (source-sorted) edges: dma_gather pulls g rows to SBUF
in 128-edge tiles (512-edge instructions round-robined over 3 SWDGE queues),
a DVE is_equal against an iota tile builds the one-hot [128 edges, 128 slots]
(stationary lhsT), and the PE accumulates S = onehot^T @ G in PSUM
[128 dst, 128 hid].  The flush scales by dinv_d (per-partition ACT scale),
adds bias, applies PReLU (relu/derived ops), and writes rows in slot order;
the host inverse-permutes slots back to node order.

int16 gather indices are made relative to a per-instruction window base
(edges sorted by source within each block keep every instruction's source
span well under 32768 rows).
"""
import sys
import time
sys.path.insert(0, '/opt/trn_rl_repo')

import numpy as np
import ml_dtypes

import concourse.bass as bass
import concourse.bacc as bacc
import concourse.mybir as mybir
import concourse.tile as tile
from concourse.bass_utils import run_bass_kernel_spmd

N_NODES = 100000
N_EDGES = 1600000
N_FEAT = 256
HIDDEN = 128

P = 128
NCORES = 8
BLK = 128                      # destination slots per block
NB = 784                       # blocks total (784*128 = 100352 slots)
BPC = NB // NCORES             # 49 blocks per core
NSLOT = NB * BLK               # 100352
NPC = NSLOT // NCORES          # 12544 nodes per phase-1 shard / out cols per core
WINDOW = 32768                 # int16 gather index range
NQUEUES = 3                    # SWDGE queues (queue 3 is broken on this HW)
MAX_IDX_PER_INSTR = 1024       # descriptor-ring limit per dma_gather

F32 = mybir.dt.float32
BF16 = mybir.dt.bfloat16
I16 = mybir.dt.int16


def _prep(x, edge_index, W, b, prelu_a):
    """All host-side sharding/index prep. Returns (in_maps, schedule, unperm)."""
    col = edge_index[1].astype(np.int64)
    row = edge_index[0].astype(np.int64)

    deg = np.bincount(col, minlength=N_NODES).astype(np.int64) + 1
    dinv = (1.0 / np.sqrt(deg.astype(np.float64))).astype(np.float32)

    # degree-balanced destination blocks: serpentine over degree-sorted nodes
    order = np.argsort(-deg, kind='stable')
    blk_of_rank = np.arange(NSLOT) % (2 * NB)
    blk_of_rank = np.where(blk_of_rank < NB, blk_of_rank, 2 * NB - 1 - blk_of_rank)
    node_block = np.full(NSLOT, -1, dtype=np.int64)   # block id per sorted rank
    # assign real nodes to blocks by serpentine; dummy slots fill the rest
    node_of_rank = np.concatenate([order, np.full(NSLOT - N_NODES, -1, np.int64)])
    block_of = np.zeros(N_NODES, dtype=np.int64)
    slot_in_block = np.zeros(N_NODES, dtype=np.int64)
    fill = np.zeros(NB, dtype=np.int64)
    blk_ids = blk_of_rank[:N_NODES]
    # slot index = running count per block over the serpentine sequence
    for bball in range(NB):
        m = blk_ids == bball
        nodes = order[m]
        block_of[nodes] = bball
        slot_in_block[nodes] = np.arange(nodes.shape[0])
        fill[bball] = nodes.shape[0]
    assert fill.max() <= BLK

    # edges + self loops, grouped by destination block, sorted by source row
    erow = np.concatenate([row, np.arange(N_NODES, dtype=np.int64)])
    ecol = np.concatenate([col, np.arange(N_NODES, dtype=np.int64)])
    eblk = block_of[ecol]
    eslot = slot_in_block[ecol]
    sort = np.lexsort((erow, eblk))
    erow, eblk, eslot = erow[sort], eblk[sort], eslot[sort]

    counts = np.bincount(eblk, minlength=NB)
    T = int(np.ceil(counts.max() / P))            # tiles per block (uniform)
    cap = T * P
    starts = np.zeros(NB + 1, dtype=np.int64)
    np.cumsum(counts, out=starts[1:])

    # padded per-block edge arrays [NB, cap]: repeat last edge, slot 300
    pad_row = np.empty((NB, cap), dtype=np.int64)
    pad_slot = np.full((NB, cap), 300, dtype=np.int64)
    for bb in range(NB):
        n = counts[bb]
        s = starts[bb]
        pad_row[bb, :n] = erow[s:s + n]
        pad_slot[bb, :n] = eslot[s:s + n]
        pad_row[bb, n:] = erow[s + n - 1]

    # instruction split of T tiles per block slot: 512-edge instructions keep
    # each instruction's sorted-source span within the int16 window; merge the
    # trailing remainder into the last instruction when the span still fits.
    def splits_for(j, sizes):
        tiles = np.asarray(sizes)
        idx0 = np.concatenate([[0], np.cumsum(tiles[:-1])]) * P
        nidx = tiles * P
        bs = np.zeros(len(sizes), dtype=np.int64)
        blocks = j + BPC * np.arange(NCORES)
        for k in range(len(sizes)):
            a = idx0[k]
            lo = pad_row[blocks, a].min()
            hi = pad_row[blocks, a + nidx[k] - 1].max()
            if hi - lo >= WINDOW:
                return None
            bs[k] = min(lo, NSLOT - WINDOW)
        return tiles, idx0, nidx, bs

    CAPT = 2   # tiles per gather instruction (256 idxs)
    base_sizes = [CAPT] * (T // CAPT) + ([T % CAPT] if T % CAPT else [])
    merged_sizes = None
    if len(base_sizes) >= 2 and base_sizes[-1] < 4:
        merged_sizes = base_sizes[:-2] + [base_sizes[-2] + base_sizes[-1]]
    per_block = []
    for j in range(BPC):
        r = None
        if merged_sizes is not None:
            r = splits_for(j, merged_sizes)
        if r is None:
            r = splits_for(j, base_sizes)
            assert r is not None, j
        per_block.append(r)

    # device arrays per core
    tot_idx = BPC * cap
    in_maps = []
    node_of_slot = np.full(NSLOT, -1, dtype=np.int64)
    for bb in range(NB):
        m = block_of == bb
        nodes = np.nonzero(m)[0]
        node_of_slot[bb * BLK + slot_in_block[nodes]] = nodes

    xT = np.zeros((N_FEAT, NSLOT), dtype=np.float32)
    xT[:, :N_NODES] = np.asarray(x, dtype=np.float32).T
    dinv_pad = np.zeros(NSLOT, dtype=np.float32)
    dinv_pad[:N_NODES] = dinv

    iota = np.tile(np.arange(BLK, dtype=ml_dtypes.bfloat16), (P, 1))
    b_bc = np.tile(np.asarray(b, np.float32).reshape(1, HIDDEN), (P, 1))
    a_bc = np.tile(np.asarray(prelu_a, np.float32).reshape(1, HIDDEN), (P, 1))
    wt = np.ascontiguousarray(np.asarray(W, np.float32).T)      # [256, 128]

    for c in range(NCORES):
        blocks = c * BPC + np.arange(BPC)
        rows_c = pad_row[blocks]          # [BPC, cap]
        slots_c = pad_slot[blocks]        # [BPC, cap]

        # int16 idx: per (block j, instr k): edge i -> arr16[i%16, i//16]
        idx16 = np.empty((16, tot_idx // 16), dtype=np.int16)
        # localcol: [128, BPC*T]: tile t, partition p -> edge t*128+p
        lcol = np.empty((P, BPC * T), dtype=ml_dtypes.bfloat16)
        off16 = 0
        for j in range(BPC):
            r = rows_c[j]
            tiles_j, idx0_j, nidx_j, bases_j = per_block[j]
            for k in range(len(tiles_j)):
                a = idx0_j[k]
                nidx = nidx_j[k]
                rel = r[a:a + nidx] - bases_j[k]
                assert rel.min() >= 0 and rel.max() < WINDOW, (c, j, k)
                idx16[:, off16:off16 + nidx // 16] = \
                    rel.reshape(nidx // 16, 16).T.astype(np.int16)
                off16 += nidx // 16
            lcol[:, j * T:(j + 1) * T] = slots_c[j].reshape(T, P).T
        idx_full = np.tile(idx16, (8, 1))

        # dinv of each destination slot: [128 slot-in-block, BPC]
        nos = node_of_slot[c * NPC:(c + 1) * NPC]
        dslot = dinv_pad[nos % N_NODES] * (nos >= 0)
        dinv_slot = dslot.astype(np.float32).reshape(BPC, P).T.copy()

        dinv_node = dinv_pad[c * NPC:(c + 1) * NPC].reshape(NPC // P, P).T.copy()

        in_maps.append({
            "xt": np.ascontiguousarray(xT[:, c * NPC:(c + 1) * NPC]),
            "wt": wt,
            "dinv_node": dinv_node,        # [128, 98]
            "idxs": idx_full,              # [128, tot_idx//16] int16
            "lcol": lcol,                  # [128, BPC*T] bf16
            "dinv_slot": dinv_slot,        # [128, BPC]
            "iota": iota,                  # [128, 128] bf16
            "b_bc": b_bc, "a_bc": a_bc,    # [128, 128]
        })

    sched = dict(T=T, per_block=tuple(
        (tuple(int(v) for v in pb_[0]), tuple(int(v) for v in pb_[3]))
        for pb_ in per_block), tot_idx=tot_idx)
    return in_maps, sched, node_of_slot


def _build(sched):
    T = sched["T"]
    per_block = sched["per_block"]
    tot_idx = sched["tot_idx"]

    nc = bacc.Bacc("TRN2", target_bir_lowering=False, debug=False,
                   num_devices=NCORES, num_swdge_queues=NQUEUES)

    xt = nc.dram_tensor("xt", [N_FEAT, NPC], F32, kind="ExternalInput").ap()
    wt = nc.dram_tensor("wt", [N_FEAT, HIDDEN], F32, kind="ExternalInput").ap()
    dinv_node = nc.dram_tensor("dinv_node", [P, NPC // P], F32, kind="ExternalInput").ap()
    idxs = nc.dram_tensor("idxs", [P, tot_idx // 16], I16, kind="ExternalInput").ap()
    lcolt = nc.dram_tensor("lcol", [P, (NB // NCORES) * T], BF16, kind="ExternalInput")
    dinv_slot = nc.dram_tensor("dinv_slot", [P, NB // NCORES], F32, kind="ExternalInput").ap()
    iota = nc.dram_tensor("iota", [P, BLK], BF16, kind="ExternalInput").ap()
    b_bc = nc.dram_tensor("b_bc", [P, HIDDEN], F32, kind="ExternalInput").ap()
    a_bc = nc.dram_tensor("a_bc", [P, HIDDEN], F32, kind="ExternalInput").ap()
    out_d = nc.dram_tensor("out_d", [NPC, HIDDEN], F32, kind="ExternalOutput").ap()

    with tile.TileContext(nc) as tc:
        with (
            tc.tile_pool(name="dram", bufs=1, space="DRAM") as dram,
            tc.tile_pool(name="const", bufs=1) as cp,
            tc.tile_pool(name="x", bufs=3) as xp,
            tc.tile_pool(name="g", bufs=3) as gp,
            tc.tile_pool(name="ph1psum", bufs=3, space="PSUM") as pp1,
            tc.tile_pool(name="gat", bufs=12) as gat,
            tc.tile_pool(name="oh", bufs=8) as ohp,
            tc.tile_pool(name="fl", bufs=3) as fl,
            tc.tile_pool(name="ph2psum", bufs=5, space="PSUM") as pp2,
        ):
            g_shard = dram.tile([NPC, HIDDEN], BF16)
            g_full = dram.tile([NSLOT, HIDDEN], BF16, addr_space="Shared")

            # constants to SBUF
            wt_sb = cp.tile([P, N_FEAT // P, HIDDEN], F32)
            nc.sync.dma_start(out=wt_sb[:], in_=wt.rearrange("(a p) h -> p a h", p=P))
            dinv_sb = cp.tile([P, NPC // P], F32)
            nc.sync.dma_start(out=dinv_sb[:], in_=dinv_node)
            # ---- phase 1: g_shard = (xT_c.T @ W.T) * dinv, cast to bf16 ----
            NT1 = NPC // P                       # 98 node tiles
            GRP = 7                              # node tiles per load group
            for gi in range(NT1 // GRP):
                xbuf = xp.tile([P, 2, GRP * P], F32, tag="xbuf")
                nc.sync.dma_start(
                    out=xbuf[:],
                    in_=xt.rearrange("(a p) n -> p a n", p=P)[
                        :, :, gi * GRP * P:(gi + 1) * GRP * P],
                )
                gtile = gp.tile([P, GRP * P], BF16, tag="gtile")
                for s in range(GRP):
                    hp = pp1.tile([P, HIDDEN], F32, tag="hps")
                    for kk in range(2):
                        nc.tensor.matmul(
                            out=hp[:],
                            lhsT=xbuf[:, kk, bass.ts(s, P)],
                            rhs=wt_sb[:, kk, :],
                            start=(kk == 0), stop=(kk == 1),
                        )
                    nt = gi * GRP + s
                    nc.vector.tensor_tensor(
                        out=gtile[:, bass.ts(s, P)],
                        in0=hp[:],
                        in1=dinv_sb[:, nt:nt + 1].to_broadcast([P, HIDDEN]),
                        op=mybir.AluOpType.mult,
                    )
                nc.sync.dma_start(
                    out=g_shard[:].rearrange("(t p) h -> p t h", p=P)[
                        :, gi * GRP:(gi + 1) * GRP, :],
                    in_=gtile[:].rearrange("p (t h) -> p t h", h=HIDDEN),
                )

            # ---- all-gather the g table (Shared output: avoids the extra
            # HBM bounce copy inside the collective) ----
            nc.gpsimd.collective_compute(
                "AllGather",
                mybir.AluOpType.bypass,
                ins=[g_shard[:].opt()],
                outs=[g_full[:].opt()],
                replica_groups=[list(range(NCORES))],
            )

            # phase-2 constants: issued after phase 1 so they don't delay it;
            # the DMA engines are idle during the AllGather anyway
            idx_sb = cp.tile([P, tot_idx // 16], I16)
            nc.sync.dma_start(out=idx_sb[:], in_=idxs)
            lcol_sb = cp.tile([P, BPC * T], BF16)
            nc.sync.dma_start(out=lcol_sb[:], in_=lcolt.ap())
            dslot_sb = cp.tile([P, BPC], F32)
            nc.sync.dma_start(out=dslot_sb[:], in_=dinv_slot)
            iota_sb = cp.tile([P, BLK], BF16)
            nc.sync.dma_start(out=iota_sb[:], in_=iota)
            bbc_sb = cp.tile([P, HIDDEN], F32)
            nc.sync.dma_start(out=bbc_sb[:], in_=b_bc)
            abc_sb = cp.tile([P, HIDDEN], F32)
            nc.sync.dma_start(out=abc_sb[:], in_=a_bc)

            # ---- phase 2: gather + one-hot matmul accumulate + flush ----
            qn = 0
            off16 = 0
            for j in range(BPC):
                ps = pp2.tile([P, BLK], F32, tag="ps")
                tiles_j, bases_j = per_block[j]
                KI = len(tiles_j)
                tglob = j * T
                tile_in_blk = 0
                for k in range(KI):
                    ntl = int(tiles_j[k])
                    nidx = ntl * P
                    base = int(bases_j[k])
                    chunk = gat.tile([P, 5 * HIDDEN], BF16, tag="chunk")
                    c3 = chunk[:].rearrange("p (t h) -> p t h", h=HIDDEN)
                    nc.gpsimd.dma_gather(
                        c3[:, :ntl, :],
                        g_full[:][base:base + WINDOW, :],
                        idx_sb[:, off16:off16 + nidx // 16],
                        nidx, nidx, HIDDEN,
                        queue_num=qn,
                    )
                    qn = (qn + 1) % NQUEUES
                    off16 += nidx // 16
                    for t in range(ntl):
                        oh = ohp.tile([P, BLK], BF16, tag="oh")
                        tt = tglob + tile_in_blk
                        tile_in_blk += 1
                        nc.vector.tensor_tensor(
                            out=oh[:], in0=iota_sb[:],
                            in1=lcol_sb[:, tt:tt + 1].to_broadcast([P, BLK]),
                            op=mybir.AluOpType.is_equal,
                        )
                        nc.tensor.matmul(
                            out=ps[:],
                            lhsT=oh[:],
                            rhs=c3[:, t, :],
                            start=(tile_in_blk == 1),
                            stop=(tile_in_blk == T),
                        )
                # flush block j: out = prelu(dinv_d * S + b), dst on partitions
                u = fl.tile([P, HIDDEN], F32, tag="u")
                nc.scalar.activation(out=u[:], in_=ps[:],
                                     func=mybir.ActivationFunctionType.Copy,
                                     bias=0.0, scale=dslot_sb[:, j:j + 1])
                v = fl.tile([P, HIDDEN], F32, tag="v")
                nc.vector.tensor_tensor(out=v[:], in0=u[:], in1=bbc_sb[:],
                                        op=mybir.AluOpType.add)
                pos = fl.tile([P, HIDDEN], F32, tag="pos")
                nc.scalar.activation(out=pos[:], in_=v[:],
                                     func=mybir.ActivationFunctionType.Relu,
                                     bias=0.0, scale=1.0)
                neg = fl.tile([P, HIDDEN], F32, tag="neg")
                nc.vector.tensor_tensor(out=neg[:], in0=v[:], in1=pos[:],
                                        op=mybir.AluOpType.subtract)
                res = fl.tile([P, HIDDEN], F32, tag="res")
                nc.vector.tensor_tensor(out=res[:], in0=neg[:], in1=abc_sb[:],
                                        op=mybir.AluOpType.mult)
                res2 = fl.tile([P, HIDDEN], F32, tag="res2")
                nc.vector.tensor_tensor(out=res2[:], in0=pos[:], in1=res[:],
                                        op=mybir.AluOpType.add)
                nc.sync.dma_start(out=out_d[j * BLK:(j + 1) * BLK, :], in_=res2[:])

    nc.finalize()
    return nc


_CACHED = {}


def kernel(x, edge_index, W, b, prelu_a):
    x = np.asarray(x)
    edge_index = np.asarray(edge_index)
    W = np.asarray(W)
    b = np.asarray(b)
    prelu_a = np.asarray(prelu_a)

    in_maps, sched, node_of_slot = _prep(x, edge_index, W, b, prelu_a)
    key = (sched["T"], sched["per_block"])
    if key not in _CACHED:
        _CACHED[key] = _build(sched)
    nc = _CACHED[key]

    last_err = None
    for attempt in range(3):
        try:
            res = run_bass_kernel_spmd(nc, in_maps, core_ids=list(range(NCORES)))
            break
        except Exception as e:          # transient NRT/device hiccups
            last_err = e
            time.sleep(2.0)
    else:
        raise last_err

    out_slots = np.concatenate(
        [res.results[c]["out_d"] for c in range(NCORES)], axis=0)  # [NSLOT, H]
    out = np.zeros((N_NODES, HIDDEN), dtype=np.float32)
    valid = node_of_slot >= 0
    out[node_of_slot[valid]] = out_slots[valid]
    return out


# revision 19
# speedup vs baseline: 1.1687x; 1.1687x over previous
"""GCN layer (GCNConv + PReLU) on 8 Trainium2 NeuronCores.

Math (equivalent to the PyG-style reference):
    h   = x @ W.T                       # [N, H] dense transform
    deg = bincount(col) + 1             # self-loops included
    dinv = 1/sqrt(deg)
    g   = h * dinv[:, None]             # fold source-side norm into nodes
    S_d = sum_{e: col_e = d} g[row_e]   # includes self edge (d, d)
    out = prelu(dinv_d * S_d + b)       # per-channel slope a

Distribution: destination nodes are grouped into 784 degree-balanced blocks
of 128 slots; each core owns 98 blocks.  Phase 1 computes g for a contiguous
node shard per core (x.T shard @ W.T on the PE, fp32), then an AllGather
replicates the full g table [100352, 128] (bf16) to every core.  Phase 2
streams each block's (source-sorted) edges: dma_gather pulls g rows to SBUF
in 128-edge tiles (512-edge instructions round-robined over 3 SWDGE queues),
a DVE is_equal against an iota tile builds the one-hot [128 edges, 128 slots]
(stationary lhsT), and the PE accumulates S = onehot^T @ G in PSUM
[128 dst, 128 hid].  The flush scales by dinv_d (per-partition ACT scale),
adds bias, applies PReLU (relu/derived ops), and writes rows in slot order;
the host inverse-permutes slots back to node order.

int16 gather indices are made relative to a per-instruction window base
(edges sorted by source within each block keep every instruction's source
span well under 32768 rows).
"""
import sys
import time
sys.path.insert(0, '/opt/trn_rl_repo')

import numpy as np
import ml_dtypes

import concourse.bass as bass
import concourse.bacc as bacc
import concourse.mybir as mybir
import concourse.tile as tile
from concourse.bass_utils import run_bass_kernel_spmd

N_NODES = 100000
N_EDGES = 1600000
N_FEAT = 256
HIDDEN = 128

P = 128
NCORES = 8
BLK = 128                      # destination slots per block
NB = 784                       # blocks total (784*128 = 100352 slots)
BPC = NB // NCORES             # 49 blocks per core
NSLOT = NB * BLK               # 100352
NPC = NSLOT // NCORES          # 12544 nodes per phase-1 shard / out cols per core
WINDOW = 32768                 # int16 gather index range
NQUEUES = 3                    # SWDGE queues (queue 3 is broken on this HW)
MAX_IDX_PER_INSTR = 1024       # descriptor-ring limit per dma_gather

F32 = mybir.dt.float32
BF16 = mybir.dt.bfloat16
I16 = mybir.dt.int16


def _prep(x, edge_index, W, b, prelu_a):
    """All host-side sharding/index prep. Returns (in_maps, schedule, unperm)."""
    col = edge_index[1].astype(np.int64)
    row = edge_index[0].astype(np.int64)

    deg = np.bincount(col, minlength=N_NODES).astype(np.int64) + 1
    dinv = (1.0 / np.sqrt(deg.astype(np.float64))).astype(np.float32)

    # degree-balanced destination blocks: serpentine over degree-sorted nodes
    order = np.argsort(-deg, kind='stable')
    blk_of_rank = np.arange(NSLOT) % (2 * NB)
    blk_of_rank = np.where(blk_of_rank < NB, blk_of_rank, 2 * NB - 1 - blk_of_rank)
    node_block = np.full(NSLOT, -1, dtype=np.int64)   # block id per sorted rank
    # assign real nodes to blocks by serpentine; dummy slots fill the rest
    node_of_rank = np.concatenate([order, np.full(NSLOT - N_NODES, -1, np.int64)])
    block_of = np.zeros(N_NODES, dtype=np.int64)
    slot_in_block = np.zeros(N_NODES, dtype=np.int64)
    fill = np.zeros(NB, dtype=np.int64)
    blk_ids = blk_of_rank[:N_NODES]
    # slot index = running count per block over the serpentine sequence
    for bball in range(NB):
        m = blk_ids == bball
        nodes = order[m]
        block_of[nodes] = bball
        slot_in_block[nodes] = np.arange(nodes.shape[0])
        fill[bball] = nodes.shape[0]
    assert fill.max() <= BLK

    # edges + self loops, grouped by destination block, sorted by source row
    erow = np.concatenate([row, np.arange(N_NODES, dtype=np.int64)])
    ecol = np.concatenate([col, np.arange(N_NODES, dtype=np.int64)])
    eblk = block_of[ecol]
    eslot = slot_in_block[ecol]
    sort = np.lexsort((erow, eblk))
    erow, eblk, eslot = erow[sort], eblk[sort], eslot[sort]

    counts = np.bincount(eblk, minlength=NB)
    T = int(np.ceil(counts.max() / P))            # tiles per block (uniform)
    cap = T * P
    starts = np.zeros(NB + 1, dtype=np.int64)
    np.cumsum(counts, out=starts[1:])

    # padded per-block edge arrays [NB, cap]: repeat last edge, slot 300
    pad_row = np.empty((NB, cap), dtype=np.int64)
    pad_slot = np.full((NB, cap), 300, dtype=np.int64)
    for bb in range(NB):
        n = counts[bb]
        s = starts[bb]
        pad_row[bb, :n] = erow[s:s + n]
        pad_slot[bb, :n] = eslot[s:s + n]
        pad_row[bb, n:] = erow[s + n - 1]

    # instruction split of T tiles per block slot: 512-edge instructions keep
    # each instruction's sorted-source span within the int16 window; merge the
    # trailing remainder into the last instruction when the span still fits.
    def splits_for(j, sizes):
        tiles = np.asarray(sizes)
        idx0 = np.concatenate([[0], np.cumsum(tiles[:-1])]) * P
        nidx = tiles * P
        bs = np.zeros(len(sizes), dtype=np.int64)
        blocks = j + BPC * np.arange(NCORES)
        for k in range(len(sizes)):
            a = idx0[k]
            lo = pad_row[blocks, a].min()
            hi = pad_row[blocks, a + nidx[k] - 1].max()
            if hi - lo >= WINDOW:
                return None
            bs[k] = min(lo, NSLOT - WINDOW)
        return tiles, idx0, nidx, bs

    CAPT = 4   # tiles per gather instruction (512 idxs: best measured tradeoff
    # between per-instruction overhead and int16 window span)
    base_sizes = [CAPT] * (T // CAPT) + ([T % CAPT] if T % CAPT else [])
    merged_sizes = None
    if len(base_sizes) >= 2 and base_sizes[-1] < 4:
        merged_sizes = base_sizes[:-2] + [base_sizes[-2] + base_sizes[-1]]
    per_block = []
    for j in range(BPC):
        r = None
        if merged_sizes is not None:
            r = splits_for(j, merged_sizes)
        if r is None:
            r = splits_for(j, base_sizes)
            assert r is not None, j
        per_block.append(r)

    # device arrays per core
    tot_idx = BPC * cap
    in_maps = []
    node_of_slot = np.full(NSLOT, -1, dtype=np.int64)
    for bb in range(NB):
        m = block_of == bb
        nodes = np.nonzero(m)[0]
        node_of_slot[bb * BLK + slot_in_block[nodes]] = nodes

    xT = np.zeros((N_FEAT, NSLOT), dtype=np.float32)
    xT[:, :N_NODES] = np.asarray(x, dtype=np.float32).T
    dinv_pad = np.zeros(NSLOT, dtype=np.float32)
    dinv_pad[:N_NODES] = dinv

    iota = np.tile(np.arange(BLK, dtype=ml_dtypes.bfloat16), (P, 1))
    b_bc = np.tile(np.asarray(b, np.float32).reshape(1, HIDDEN), (P, 1))
    a_bc = np.tile(np.asarray(prelu_a, np.float32).reshape(1, HIDDEN), (P, 1))
    wt = np.ascontiguousarray(np.asarray(W, np.float32).T)      # [256, 128]

    for c in range(NCORES):
        blocks = c * BPC + np.arange(BPC)
        rows_c = pad_row[blocks]          # [BPC, cap]
        slots_c = pad_slot[blocks]        # [BPC, cap]

        # int16 idx: per (block j, instr k): edge i -> arr16[i%16, i//16]
        idx16 = np.empty((16, tot_idx // 16), dtype=np.int16)
        # localcol: [128, BPC*T]: tile t, partition p -> edge t*128+p
        lcol = np.empty((P, BPC * T), dtype=ml_dtypes.bfloat16)
        off16 = 0
        for j in range(BPC):
            r = rows_c[j]
            tiles_j, idx0_j, nidx_j, bases_j = per_block[j]
            for k in range(len(tiles_j)):
                a = idx0_j[k]
                nidx = nidx_j[k]
                rel = r[a:a + nidx] - bases_j[k]
                assert rel.min() >= 0 and rel.max() < WINDOW, (c, j, k)
                idx16[:, off16:off16 + nidx // 16] = \
                    rel.reshape(nidx // 16, 16).T.astype(np.int16)
                off16 += nidx // 16
            lcol[:, j * T:(j + 1) * T] = slots_c[j].reshape(T, P).T
        idx_full = np.tile(idx16, (8, 1))

        # dinv of each destination slot: [128 slot-in-block, BPC]
        nos = node_of_slot[c * NPC:(c + 1) * NPC]
        dslot = dinv_pad[nos % N_NODES] * (nos >= 0)
        dinv_slot = dslot.astype(np.float32).reshape(BPC, P).T.copy()

        dinv_node = dinv_pad[c * NPC:(c + 1) * NPC].reshape(NPC // P, P).T.copy()

        in_maps.append({
            "xt": np.ascontiguousarray(xT[:, c * NPC:(c + 1) * NPC]),
            "wt": wt,
            "dinv_node": dinv_node,        # [128, 98]
            "idxs": idx_full,              # [128, tot_idx//16] int16
            "lcol": lcol,                  # [128, BPC*T] bf16
            "dinv_slot": dinv_slot,        # [128, BPC]
            "iota": iota,                  # [128, 128] bf16
            "b_bc": b_bc, "a_bc": a_bc,    # [128, 128]
        })

    sched = dict(T=T, per_block=tuple(
        (tuple(int(v) for v in pb_[0]), tuple(int(v) for v in pb_[3]))
        for pb_ in per_block), tot_idx=tot_idx)
    return in_maps, sched, node_of_slot


def _build(sched):
    T = sched["T"]
    per_block = sched["per_block"]
    tot_idx = sched["tot_idx"]

    nc = bacc.Bacc("TRN2", target_bir_lowering=False, debug=False,
                   num_devices=NCORES, num_swdge_queues=NQUEUES)

    xt = nc.dram_tensor("xt", [N_FEAT, NPC], F32, kind="ExternalInput").ap()
    wt = nc.dram_tensor("wt", [N_FEAT, HIDDEN], F32, kind="ExternalInput").ap()
    dinv_node = nc.dram_tensor("dinv_node", [P, NPC // P], F32, kind="ExternalInput").ap()
    idxs = nc.dram_tensor("idxs", [P, tot_idx // 16], I16, kind="ExternalInput").ap()
    lcolt = nc.dram_tensor("lcol", [P, (NB // NCORES) * T], BF16, kind="ExternalInput")
    dinv_slot = nc.dram_tensor("dinv_slot", [P, NB // NCORES], F32, kind="ExternalInput").ap()
    iota = nc.dram_tensor("iota", [P, BLK], BF16, kind="ExternalInput").ap()
    b_bc = nc.dram_tensor("b_bc", [P, HIDDEN], F32, kind="ExternalInput").ap()
    a_bc = nc.dram_tensor("a_bc", [P, HIDDEN], F32, kind="ExternalInput").ap()
    out_d = nc.dram_tensor("out_d", [NPC, HIDDEN], F32, kind="ExternalOutput").ap()

    with tile.TileContext(nc) as tc:
        with (
            tc.tile_pool(name="dram", bufs=1, space="DRAM") as dram,
            tc.tile_pool(name="const", bufs=1) as cp,
            tc.tile_pool(name="x", bufs=3) as xp,
            tc.tile_pool(name="g", bufs=3) as gp,
            tc.tile_pool(name="ph1psum", bufs=3, space="PSUM") as pp1,
            tc.tile_pool(name="gat", bufs=8) as gat,
            tc.tile_pool(name="oh", bufs=8) as ohp,
            tc.tile_pool(name="fl", bufs=3) as fl,
            tc.tile_pool(name="ph2psum", bufs=5, space="PSUM") as pp2,
        ):
            g_shard = dram.tile([NPC, HIDDEN], BF16)
            g_full = dram.tile([NSLOT, HIDDEN], BF16, addr_space="Shared")

            # constants to SBUF
            wt_sb = cp.tile([P, N_FEAT // P, HIDDEN], F32)
            nc.sync.dma_start(out=wt_sb[:], in_=wt.rearrange("(a p) h -> p a h", p=P))
            dinv_sb = cp.tile([P, NPC // P], F32)
            nc.sync.dma_start(out=dinv_sb[:], in_=dinv_node)
            # ---- phase 1: g_shard = (xT_c.T @ W.T) * dinv, cast to bf16 ----
            NT1 = NPC // P                       # 98 node tiles
            GRP = 7                              # node tiles per load group
            for gi in range(NT1 // GRP):
                xbuf = xp.tile([P, 2, GRP * P], F32, tag="xbuf")
                nc.sync.dma_start(
                    out=xbuf[:],
                    in_=xt.rearrange("(a p) n -> p a n", p=P)[
                        :, :, gi * GRP * P:(gi + 1) * GRP * P],
                )
                gtile = gp.tile([P, GRP * P], BF16, tag="gtile")
                for s in range(GRP):
                    hp = pp1.tile([P, HIDDEN], F32, tag="hps")
                    for kk in range(2):
                        nc.tensor.matmul(
                            out=hp[:],
                            lhsT=xbuf[:, kk, bass.ts(s, P)],
                            rhs=wt_sb[:, kk, :],
                            start=(kk == 0), stop=(kk == 1),
                        )
                    nt = gi * GRP + s
                    nc.vector.tensor_tensor(
                        out=gtile[:, bass.ts(s, P)],
                        in0=hp[:],
                        in1=dinv_sb[:, nt:nt + 1].to_broadcast([P, HIDDEN]),
                        op=mybir.AluOpType.mult,
                    )
                nc.sync.dma_start(
                    out=g_shard[:].rearrange("(t p) h -> p t h", p=P)[
                        :, gi * GRP:(gi + 1) * GRP, :],
                    in_=gtile[:].rearrange("p (t h) -> p t h", h=HIDDEN),
                )

            # ---- all-gather the g table (Shared output: avoids the extra
            # HBM bounce copy inside the collective) ----
            nc.gpsimd.collective_compute(
                "AllGather",
                mybir.AluOpType.bypass,
                ins=[g_shard[:].opt()],
                outs=[g_full[:].opt()],
                replica_groups=[list(range(NCORES))],
            )

            # phase-2 constants: issued after phase 1 so they don't delay it;
            # the DMA engines are idle during the AllGather anyway
            idx_sb = cp.tile([P, tot_idx // 16], I16)
            nc.sync.dma_start(out=idx_sb[:], in_=idxs)
            lcol_sb = cp.tile([P, BPC * T], BF16)
            nc.sync.dma_start(out=lcol_sb[:], in_=lcolt.ap())
            dslot_sb = cp.tile([P, BPC], F32)
            nc.sync.dma_start(out=dslot_sb[:], in_=dinv_slot)
            iota_sb = cp.tile([P, BLK], BF16)
            nc.sync.dma_start(out=iota_sb[:], in_=iota)
            bbc_sb = cp.tile([P, HIDDEN], F32)
            nc.sync.dma_start(out=bbc_sb[:], in_=b_bc)
            abc_sb = cp.tile([P, HIDDEN], F32)
            nc.sync.dma_start(out=abc_sb[:], in_=a_bc)

            # ---- phase 2: gather + one-hot matmul accumulate + flush ----
            qn = 0
            off16 = 0
            for j in range(BPC):
                ps = pp2.tile([P, BLK], F32, tag="ps")
                tiles_j, bases_j = per_block[j]
                KI = len(tiles_j)
                tglob = j * T
                tile_in_blk = 0
                for k in range(KI):
                    ntl = int(tiles_j[k])
                    nidx = ntl * P
                    base = int(bases_j[k])
                    chunk = gat.tile([P, 5 * HIDDEN], BF16, tag="chunk")
                    c3 = chunk[:].rearrange("p (t h) -> p t h", h=HIDDEN)
                    nc.gpsimd.dma_gather(
                        c3[:, :ntl, :],
                        g_full[:][base:base + WINDOW, :],
                        idx_sb[:, off16:off16 + nidx // 16],
                        nidx, nidx, HIDDEN,
                        queue_num=qn,
                    )
                    qn = (qn + 1) % NQUEUES
                    off16 += nidx // 16
                    for t in range(ntl):
                        oh = ohp.tile([P, BLK], BF16, tag="oh")
                        tt = tglob + tile_in_blk
                        tile_in_blk += 1
                        nc.vector.tensor_tensor(
                            out=oh[:], in0=iota_sb[:],
                            in1=lcol_sb[:, tt:tt + 1].to_broadcast([P, BLK]),
                            op=mybir.AluOpType.is_equal,
                        )
                        nc.tensor.matmul(
                            out=ps[:],
                            lhsT=oh[:],
                            rhs=c3[:, t, :],
                            start=(tile_in_blk == 1),
                            stop=(tile_in_blk == T),
                        )
                # flush block j: out = prelu(dinv_d * S + b), dst on partitions
                u = fl.tile([P, HIDDEN], F32, tag="u")
                nc.scalar.activation(out=u[:], in_=ps[:],
                                     func=mybir.ActivationFunctionType.Copy,
                                     bias=0.0, scale=dslot_sb[:, j:j + 1])
                v = fl.tile([P, HIDDEN], F32, tag="v")
                nc.vector.tensor_tensor(out=v[:], in0=u[:], in1=bbc_sb[:],
                                        op=mybir.AluOpType.add)
                pos = fl.tile([P, HIDDEN], F32, tag="pos")
                nc.scalar.activation(out=pos[:], in_=v[:],
                                     func=mybir.ActivationFunctionType.Relu,
                                     bias=0.0, scale=1.0)
                neg = fl.tile([P, HIDDEN], F32, tag="neg")
                nc.vector.tensor_tensor(out=neg[:], in0=v[:], in1=pos[:],
                                        op=mybir.AluOpType.subtract)
                res = fl.tile([P, HIDDEN], F32, tag="res")
                nc.vector.tensor_tensor(out=res[:], in0=neg[:], in1=abc_sb[:],
                                        op=mybir.AluOpType.mult)
                res2 = fl.tile([P, HIDDEN], F32, tag="res2")
                nc.vector.tensor_tensor(out=res2[:], in0=pos[:], in1=res[:],
                                        op=mybir.AluOpType.add)
                nc.sync.dma_start(out=out_d[j * BLK:(j + 1) * BLK, :], in_=res2[:])

    nc.finalize()
    return nc


_CACHED = {}


def kernel(x, edge_index, W, b, prelu_a):
    x = np.asarray(x)
    edge_index = np.asarray(edge_index)
    W = np.asarray(W)
    b = np.asarray(b)
    prelu_a = np.asarray(prelu_a)

    in_maps, sched, node_of_slot = _prep(x, edge_index, W, b, prelu_a)
    key = (sched["T"], sched["per_block"])
    if key not in _CACHED:
        _CACHED[key] = _build(sched)
    nc = _CACHED[key]

    last_err = None
    for attempt in range(3):
        try:
            res = run_bass_kernel_spmd(nc, in_maps, core_ids=list(range(NCORES)))
            break
        except Exception as e:          # transient NRT/device hiccups
            last_err = e
            time.sleep(2.0)
    else:
        raise last_err

    out_slots = np.concatenate(
        [res.results[c]["out_d"] for c in range(NCORES)], axis=0)  # [NSLOT, H]
    out = np.zeros((N_NODES, HIDDEN), dtype=np.float32)
    valid = node_of_slot >= 0
    out[node_of_slot[valid]] = out_slots[valid]
    return out


# revision 20
# speedup vs baseline: 1.2130x; 1.0380x over previous
"""GCN layer (GCNConv + PReLU) on 8 Trainium2 NeuronCores.

Math (equivalent to the PyG-style reference):
    h   = x @ W.T                       # [N, H] dense transform
    deg = bincount(col) + 1             # self-loops included
    dinv = 1/sqrt(deg)
    g   = h * dinv[:, None]             # fold source-side norm into nodes
    S_d = sum_{e: col_e = d} g[row_e]   # includes self edge (d, d)
    out = prelu(dinv_d * S_d + b)       # per-channel slope a

Distribution: destination nodes are grouped into 784 degree-balanced blocks
of 128 slots; each core owns 98 blocks.  Phase 1 computes g for a contiguous
node shard per core (x.T shard @ W.T on the PE, fp32), then an AllGather
replicates the full g table [100352, 128] (bf16) to every core.  Phase 2
streams each block's (source-sorted) edges: dma_gather pulls g rows to SBUF
in 128-edge tiles (512-edge instructions round-robined over 3 SWDGE queues),
a DVE is_equal against an iota tile builds the one-hot [128 edges, 128 slots]
(stationary lhsT), and the PE accumulates S = onehot^T @ G in PSUM
[128 dst, 128 hid].  The flush scales by dinv_d (per-partition ACT scale),
adds bias, applies PReLU (relu/derived ops), and writes rows in slot order;
the host inverse-permutes slots back to node order.

int16 gather indices are made relative to a per-instruction window base
(edges sorted by source within each block keep every instruction's source
span well under 32768 rows).
"""
import sys
import time
sys.path.insert(0, '/opt/trn_rl_repo')

import numpy as np
import ml_dtypes

import concourse.bass as bass
import concourse.bacc as bacc
import concourse.mybir as mybir
import concourse.tile as tile
from concourse.bass_utils import run_bass_kernel_spmd

N_NODES = 100000
N_EDGES = 1600000
N_FEAT = 256
HIDDEN = 128

P = 128
NCORES = 8
BLK = 128                      # destination slots per block
NB = 784                       # blocks total (784*128 = 100352 slots)
BPC = NB // NCORES             # 49 blocks per core
NSLOT = NB * BLK               # 100352
NPC = NSLOT // NCORES          # 12544 nodes per phase-1 shard / out cols per core
WINDOW = 32768                 # int16 gather index range
NQUEUES = 3                    # SWDGE queues (queue 3 is broken on this HW)
MAX_IDX_PER_INSTR = 1024       # descriptor-ring limit per dma_gather

F32 = mybir.dt.float32
BF16 = mybir.dt.bfloat16
I16 = mybir.dt.int16


def _prep(x, edge_index, W, b, prelu_a):
    """All host-side sharding/index prep. Returns (in_maps, schedule, unperm)."""
    col = edge_index[1].astype(np.int64)
    row = edge_index[0].astype(np.int64)

    deg = np.bincount(col, minlength=N_NODES).astype(np.int64) + 1
    dinv = (1.0 / np.sqrt(deg.astype(np.float64))).astype(np.float32)

    # degree-balanced destination blocks: serpentine over degree-sorted nodes
    order = np.argsort(-deg, kind='stable')
    blk_of_rank = np.arange(NSLOT) % (2 * NB)
    blk_of_rank = np.where(blk_of_rank < NB, blk_of_rank, 2 * NB - 1 - blk_of_rank)
    node_block = np.full(NSLOT, -1, dtype=np.int64)   # block id per sorted rank
    # assign real nodes to blocks by serpentine; dummy slots fill the rest
    node_of_rank = np.concatenate([order, np.full(NSLOT - N_NODES, -1, np.int64)])
    block_of = np.zeros(N_NODES, dtype=np.int64)
    slot_in_block = np.zeros(N_NODES, dtype=np.int64)
    fill = np.zeros(NB, dtype=np.int64)
    blk_ids = blk_of_rank[:N_NODES]
    # slot index = running count per block over the serpentine sequence
    for bball in range(NB):
        m = blk_ids == bball
        nodes = order[m]
        block_of[nodes] = bball
        slot_in_block[nodes] = np.arange(nodes.shape[0])
        fill[bball] = nodes.shape[0]
    assert fill.max() <= BLK

    # edges + self loops, grouped by destination block, sorted by source row
    erow = np.concatenate([row, np.arange(N_NODES, dtype=np.int64)])
    ecol = np.concatenate([col, np.arange(N_NODES, dtype=np.int64)])
    eblk = block_of[ecol]
    eslot = slot_in_block[ecol]
    sort = np.lexsort((erow, eblk))
    erow, eblk, eslot = erow[sort], eblk[sort], eslot[sort]

    counts = np.bincount(eblk, minlength=NB)
    T = int(np.ceil(counts.max() / P))            # tiles per block (uniform)
    cap = T * P
    starts = np.zeros(NB + 1, dtype=np.int64)
    np.cumsum(counts, out=starts[1:])

    # padded per-block edge arrays [NB, cap]: repeat last edge, slot 300
    pad_row = np.empty((NB, cap), dtype=np.int64)
    pad_slot = np.full((NB, cap), 300, dtype=np.int64)
    for bb in range(NB):
        n = counts[bb]
        s = starts[bb]
        pad_row[bb, :n] = erow[s:s + n]
        pad_slot[bb, :n] = eslot[s:s + n]
        pad_row[bb, n:] = erow[s + n - 1]

    # instruction split of T tiles per block slot: 512-edge instructions keep
    # each instruction's sorted-source span within the int16 window; merge the
    # trailing remainder into the last instruction when the span still fits.
    def splits_for(j, sizes):
        tiles = np.asarray(sizes)
        idx0 = np.concatenate([[0], np.cumsum(tiles[:-1])]) * P
        nidx = tiles * P
        bs = np.zeros(len(sizes), dtype=np.int64)
        blocks = j + BPC * np.arange(NCORES)
        for k in range(len(sizes)):
            a = idx0[k]
            lo = pad_row[blocks, a].min()
            hi = pad_row[blocks, a + nidx[k] - 1].max()
            if hi - lo >= WINDOW:
                return None
            bs[k] = min(lo, NSLOT - WINDOW)
        return tiles, idx0, nidx, bs

    CAPT = 4   # tiles per gather instruction (512 idxs: best measured tradeoff
    # between per-instruction overhead and int16 window span)
    base_sizes = [CAPT] * (T // CAPT) + ([T % CAPT] if T % CAPT else [])
    merged_sizes = None
    if len(base_sizes) >= 2 and base_sizes[-1] < 4:
        merged_sizes = base_sizes[:-2] + [base_sizes[-2] + base_sizes[-1]]
    per_block = []
    for j in range(BPC):
        r = None
        if merged_sizes is not None:
            r = splits_for(j, merged_sizes)
        if r is None:
            r = splits_for(j, base_sizes)
            assert r is not None, j
        per_block.append(r)

    # device arrays per core
    tot_idx = BPC * cap
    in_maps = []
    node_of_slot = np.full(NSLOT, -1, dtype=np.int64)
    for bb in range(NB):
        m = block_of == bb
        nodes = np.nonzero(m)[0]
        node_of_slot[bb * BLK + slot_in_block[nodes]] = nodes

    xT = np.zeros((N_FEAT, NSLOT), dtype=ml_dtypes.bfloat16)
    xT[:, :N_NODES] = np.asarray(x, dtype=np.float32).T.astype(ml_dtypes.bfloat16)
    dinv_pad = np.zeros(NSLOT, dtype=np.float32)
    dinv_pad[:N_NODES] = dinv

    iota = np.tile(np.arange(BLK, dtype=ml_dtypes.bfloat16), (P, 1))
    b_bc = np.tile(np.asarray(b, np.float32).reshape(1, HIDDEN), (P, 1))
    a_bc = np.tile(np.asarray(prelu_a, np.float32).reshape(1, HIDDEN), (P, 1))
    wt = np.ascontiguousarray(
        np.asarray(W, np.float32).T.astype(ml_dtypes.bfloat16))  # [256, 128]

    for c in range(NCORES):
        blocks = c * BPC + np.arange(BPC)
        rows_c = pad_row[blocks]          # [BPC, cap]
        slots_c = pad_slot[blocks]        # [BPC, cap]

        # int16 idx: per (block j, instr k): edge i -> arr16[i%16, i//16]
        idx16 = np.empty((16, tot_idx // 16), dtype=np.int16)
        # localcol: [128, BPC*T]: tile t, partition p -> edge t*128+p
        lcol = np.empty((P, BPC * T), dtype=ml_dtypes.bfloat16)
        off16 = 0
        for j in range(BPC):
            r = rows_c[j]
            tiles_j, idx0_j, nidx_j, bases_j = per_block[j]
            for k in range(len(tiles_j)):
                a = idx0_j[k]
                nidx = nidx_j[k]
                rel = r[a:a + nidx] - bases_j[k]
                assert rel.min() >= 0 and rel.max() < WINDOW, (c, j, k)
                idx16[:, off16:off16 + nidx // 16] = \
                    rel.reshape(nidx // 16, 16).T.astype(np.int16)
                off16 += nidx // 16
            lcol[:, j * T:(j + 1) * T] = slots_c[j].reshape(T, P).T
        idx_full = np.tile(idx16, (8, 1))

        # dinv of each destination slot: [128 slot-in-block, BPC]
        nos = node_of_slot[c * NPC:(c + 1) * NPC]
        dslot = dinv_pad[nos % N_NODES] * (nos >= 0)
        dinv_slot = dslot.astype(np.float32).reshape(BPC, P).T.copy()

        dinv_node = dinv_pad[c * NPC:(c + 1) * NPC].reshape(NPC // P, P).T.copy()

        in_maps.append({
            "xt": np.ascontiguousarray(xT[:, c * NPC:(c + 1) * NPC]),
            "wt": wt,
            "dinv_node": dinv_node,        # [128, 98]
            "idxs": idx_full,              # [128, tot_idx//16] int16
            "lcol": lcol,                  # [128, BPC*T] bf16
            "dinv_slot": dinv_slot,        # [128, BPC]
            "iota": iota,                  # [128, 128] bf16
            "b_bc": b_bc, "a_bc": a_bc,    # [128, 128]
        })

    sched = dict(T=T, per_block=tuple(
        (tuple(int(v) for v in pb_[0]), tuple(int(v) for v in pb_[3]))
        for pb_ in per_block), tot_idx=tot_idx)
    return in_maps, sched, node_of_slot


def _build(sched):
    T = sched["T"]
    per_block = sched["per_block"]
    tot_idx = sched["tot_idx"]

    nc = bacc.Bacc("TRN2", target_bir_lowering=False, debug=False,
                   num_devices=NCORES, num_swdge_queues=NQUEUES)

    xt = nc.dram_tensor("xt", [N_FEAT, NPC], BF16, kind="ExternalInput").ap()
    wt = nc.dram_tensor("wt", [N_FEAT, HIDDEN], BF16, kind="ExternalInput").ap()
    dinv_node = nc.dram_tensor("dinv_node", [P, NPC // P], F32, kind="ExternalInput").ap()
    idxs = nc.dram_tensor("idxs", [P, tot_idx // 16], I16, kind="ExternalInput").ap()
    lcolt = nc.dram_tensor("lcol", [P, (NB // NCORES) * T], BF16, kind="ExternalInput")
    dinv_slot = nc.dram_tensor("dinv_slot", [P, NB // NCORES], F32, kind="ExternalInput").ap()
    iota = nc.dram_tensor("iota", [P, BLK], BF16, kind="ExternalInput").ap()
    b_bc = nc.dram_tensor("b_bc", [P, HIDDEN], F32, kind="ExternalInput").ap()
    a_bc = nc.dram_tensor("a_bc", [P, HIDDEN], F32, kind="ExternalInput").ap()
    out_d = nc.dram_tensor("out_d", [NPC, HIDDEN], F32, kind="ExternalOutput").ap()

    with tile.TileContext(nc) as tc:
        with (
            tc.tile_pool(name="dram", bufs=1, space="DRAM") as dram,
            tc.tile_pool(name="const", bufs=1) as cp,
            tc.tile_pool(name="x", bufs=3) as xp,
            tc.tile_pool(name="g", bufs=3) as gp,
            tc.tile_pool(name="ph1psum", bufs=3, space="PSUM") as pp1,
            tc.tile_pool(name="gat", bufs=8) as gat,
            tc.tile_pool(name="oh", bufs=8) as ohp,
            tc.tile_pool(name="fl", bufs=3) as fl,
            tc.tile_pool(name="ph2psum", bufs=5, space="PSUM") as pp2,
        ):
            g_shard = dram.tile([NPC, HIDDEN], BF16)
            g_full = dram.tile([NSLOT, HIDDEN], BF16, addr_space="Shared")

            # constants to SBUF
            wt_sb = cp.tile([P, N_FEAT // P, HIDDEN], BF16)
            nc.sync.dma_start(out=wt_sb[:], in_=wt.rearrange("(a p) h -> p a h", p=P))
            dinv_sb = cp.tile([P, NPC // P], F32)
            nc.sync.dma_start(out=dinv_sb[:], in_=dinv_node)
            # ---- phase 1: g_shard = (xT_c.T @ W.T) * dinv, cast to bf16 ----
            NT1 = NPC // P                       # 98 node tiles
            GRP = 7                              # node tiles per load group
            for gi in range(NT1 // GRP):
                xbuf = xp.tile([P, 2, GRP * P], BF16, tag="xbuf")
                nc.sync.dma_start(
                    out=xbuf[:],
                    in_=xt.rearrange("(a p) n -> p a n", p=P)[
                        :, :, gi * GRP * P:(gi + 1) * GRP * P],
                )
                gtile = gp.tile([P, GRP * P], BF16, tag="gtile")
                for s in range(GRP):
                    hp = pp1.tile([P, HIDDEN], F32, tag="hps")
                    for kk in range(2):
                        nc.tensor.matmul(
                            out=hp[:],
                            lhsT=xbuf[:, kk, bass.ts(s, P)],
                            rhs=wt_sb[:, kk, :],
                            start=(kk == 0), stop=(kk == 1),
                        )
                    nt = gi * GRP + s
                    nc.vector.tensor_tensor(
                        out=gtile[:, bass.ts(s, P)],
                        in0=hp[:],
                        in1=dinv_sb[:, nt:nt + 1].to_broadcast([P, HIDDEN]),
                        op=mybir.AluOpType.mult,
                    )
                nc.sync.dma_start(
                    out=g_shard[:].rearrange("(t p) h -> p t h", p=P)[
                        :, gi * GRP:(gi + 1) * GRP, :],
                    in_=gtile[:].rearrange("p (t h) -> p t h", h=HIDDEN),
                )

            # ---- all-gather the g table (Shared output: avoids the extra
            # HBM bounce copy inside the collective) ----
            nc.gpsimd.collective_compute(
                "AllGather",
                mybir.AluOpType.bypass,
                ins=[g_shard[:].opt()],
                outs=[g_full[:].opt()],
                replica_groups=[list(range(NCORES))],
            )

            # phase-2 constants: issued after phase 1 so they don't delay it;
            # the DMA engines are idle during the AllGather anyway
            idx_sb = cp.tile([P, tot_idx // 16], I16)
            nc.sync.dma_start(out=idx_sb[:], in_=idxs)
            lcol_sb = cp.tile([P, BPC * T], BF16)
            nc.sync.dma_start(out=lcol_sb[:], in_=lcolt.ap())
            dslot_sb = cp.tile([P, BPC], F32)
            nc.sync.dma_start(out=dslot_sb[:], in_=dinv_slot)
            iota_sb = cp.tile([P, BLK], BF16)
            nc.sync.dma_start(out=iota_sb[:], in_=iota)
            bbc_sb = cp.tile([P, HIDDEN], F32)
            nc.sync.dma_start(out=bbc_sb[:], in_=b_bc)
            abc_sb = cp.tile([P, HIDDEN], F32)
            nc.sync.dma_start(out=abc_sb[:], in_=a_bc)

            # ---- phase 2: gather + one-hot matmul accumulate + flush ----
            qn = 0
            off16 = 0
            for j in range(BPC):
                ps = pp2.tile([P, BLK], F32, tag="ps")
                tiles_j, bases_j = per_block[j]
                KI = len(tiles_j)
                tglob = j * T
                tile_in_blk = 0
                for k in range(KI):
                    ntl = int(tiles_j[k])
                    nidx = ntl * P
                    base = int(bases_j[k])
                    chunk = gat.tile([P, 5 * HIDDEN], BF16, tag="chunk")
                    c3 = chunk[:].rearrange("p (t h) -> p t h", h=HIDDEN)
                    nc.gpsimd.dma_gather(
                        c3[:, :ntl, :],
                        g_full[:][base:base + WINDOW, :],
                        idx_sb[:, off16:off16 + nidx // 16],
                        nidx, nidx, HIDDEN,
                        queue_num=qn,
                    )
                    qn = (qn + 1) % NQUEUES
                    off16 += nidx // 16
                    for t in range(ntl):
                        oh = ohp.tile([P, BLK], BF16, tag="oh")
                        tt = tglob + tile_in_blk
                        tile_in_blk += 1
                        nc.vector.tensor_tensor(
                            out=oh[:], in0=iota_sb[:],
                            in1=lcol_sb[:, tt:tt + 1].to_broadcast([P, BLK]),
                            op=mybir.AluOpType.is_equal,
                        )
                        nc.tensor.matmul(
                            out=ps[:],
                            lhsT=oh[:],
                            rhs=c3[:, t, :],
                            start=(tile_in_blk == 1),
                            stop=(tile_in_blk == T),
                        )
                # flush block j: out = prelu(dinv_d * S + b), dst on partitions
                u = fl.tile([P, HIDDEN], F32, tag="u")
                nc.scalar.activation(out=u[:], in_=ps[:],
                                     func=mybir.ActivationFunctionType.Copy,
                                     bias=0.0, scale=dslot_sb[:, j:j + 1])
                v = fl.tile([P, HIDDEN], F32, tag="v")
                nc.vector.tensor_tensor(out=v[:], in0=u[:], in1=bbc_sb[:],
                                        op=mybir.AluOpType.add)
                pos = fl.tile([P, HIDDEN], F32, tag="pos")
                nc.scalar.activation(out=pos[:], in_=v[:],
                                     func=mybir.ActivationFunctionType.Relu,
                                     bias=0.0, scale=1.0)
                neg = fl.tile([P, HIDDEN], F32, tag="neg")
                nc.vector.tensor_tensor(out=neg[:], in0=v[:], in1=pos[:],
                                        op=mybir.AluOpType.subtract)
                res = fl.tile([P, HIDDEN], F32, tag="res")
                nc.vector.tensor_tensor(out=res[:], in0=neg[:], in1=abc_sb[:],
                                        op=mybir.AluOpType.mult)
                res2 = fl.tile([P, HIDDEN], F32, tag="res2")
                nc.vector.tensor_tensor(out=res2[:], in0=pos[:], in1=res[:],
                                        op=mybir.AluOpType.add)
                nc.sync.dma_start(out=out_d[j * BLK:(j + 1) * BLK, :], in_=res2[:])

    nc.finalize()
    return nc


_CACHED = {}


def kernel(x, edge_index, W, b, prelu_a):
    x = np.asarray(x)
    edge_index = np.asarray(edge_index)
    W = np.asarray(W)
    b = np.asarray(b)
    prelu_a = np.asarray(prelu_a)

    in_maps, sched, node_of_slot = _prep(x, edge_index, W, b, prelu_a)
    key = (sched["T"], sched["per_block"])
    if key not in _CACHED:
        _CACHED[key] = _build(sched)
    nc = _CACHED[key]

    last_err = None
    for attempt in range(3):
        try:
            res = run_bass_kernel_spmd(nc, in_maps, core_ids=list(range(NCORES)))
            break
        except Exception as e:          # transient NRT/device hiccups
            last_err = e
            time.sleep(2.0)
    else:
        raise last_err

    out_slots = np.concatenate(
        [res.results[c]["out_d"] for c in range(NCORES)], axis=0)  # [NSLOT, H]
    out = np.zeros((N_NODES, HIDDEN), dtype=np.float32)
    valid = node_of_slot >= 0
    out[node_of_slot[valid]] = out_slots[valid]
    return out
